# revision 35
# baseline (speedup 1.0000x reference)
"""Multi-head attention block (B=2, S=2048, D=1024, H=16) on 8 trn2 cores.

Sharding: core c = (batch b = c//4, head-group g = c%4); each core computes
4 heads of one batch (Megatron column-shard of wq/wk/wv, row-shard of wo,
combined with data-parallel over batch). Host sums the 4 partial outputs
per batch and adds the (folded) bias.

v4: all-bf16, single continuously-fed PE stream.

Numerics: the attention output is a near-uniform average of ~2k value
rows, so per-element quantization error lands ~1:1 on the final output;
fp8 e4m3 (~3%/stage) blows the 2e-2 budget while bf16 (~0.1%) is free.
All matmuls run bf16 (1 col/cycle) except the f32r out-projection.

Performance: the TRN2 PE clock ramps — 2.4 GHz only after ~3us of
continuous execution, 1.2 GHz after any stall. The whole kernel is
emitted as one interleaved PE stream that never idles:
  - scores use per-head zero-PADDED K tiles [128, S] so the contraction
    is full 128 partitions (64-partition bf16 matmuls stream ~2x slower)
    with the 2-head QT tile as moving operand (the other head's rows are
    zeroed in K, so its contribution vanishes)
  - causal masking is a post-exp multiply by a shared 0/1 triangle on
    GPSIMD (off the PE/ACT critical path); diagonal blocks stream only
    the visible column suffix
  - exp on ACT: psum f32 [128,2x512] (one j-block PAIR per head) ->
    bf16 P, scale=1/8
  - PV per j-block: stationary V [128,65] (ones column -> row-sums l)
  - projection chains (Q/K/V per sb) and out-projection tiles are
    emitted as FILLER units inside the ACT-gated attention window, so
    the PE always has queued work and stays at full clock
PSUM: S-pair tiles 2x2 banks, O (PV accum) 2, filler chains 2 = 8.
Output [D, S] bf16 per core; host sums 4 partials per batch in f32 and
adds bo_eff = bo + wo@bv.
"""

import numpy as np
import ml_dtypes

import concourse.bass as bass
import concourse.mybir as mybir
import concourse.tile as tile
from concourse import bacc
from concourse.bass_utils import run_bass_kernel_spmd

B, S, D, H = 2, 2048, 1024, 16
DK = D // H                  # 64
NCORES = 8
GROUPS = NCORES // B         # 4 head-groups
HPC = H // GROUPS            # 4 heads per core
OL = HPC * DK                # 256 local features
SB = 512                     # query-block (i) width
JB = 128                     # key-block (j) width
NSB = S // SB                # 4
NJB = S // JB                # 16
ND = D // 128                # 8 contraction blocks
NDD = ND // 2                # act DMA pairing
VS = DK + 1                  # V columns per head incl. ones column (65)

F32 = mybir.dt.float32
F32R = mybir.dt.float32r
BF16 = mybir.dt.bfloat16
EXP = mybir.ActivationFunctionType.Exp

BF = ml_dtypes.bfloat16

LAST_RUN = None  # stash of BassKernelResults for test harness inspection


def _round_f32r(a):
    """Round an f32 array to the f32r grid (top-20-bit float)."""
    a = np.ascontiguousarray(a, np.float32)
    u = a.view(np.uint32)
    u = (u + 0x7FF + ((u >> 12) & 1)) & np.uint32(0xFFFFF000)
    return u.view(np.float32)


def _classify_mask(mask2):
    """Block schedule from the boolean mask [S, S] (True = visible).

    sched[ib] = list of (jb, k0, sub_ops); k0 = first visible 128-i-sub;
    sub_ops[k] in ('v', None) | ('m', None) | ('x', bias_idx).
    bias_tiles: deduplicated [n, JB, JB] multiplicative masks
    (transposed [j, i]), 1 visible / 0 masked.
    """
    sched = []
    bias_tiles = []
    bias_keys = {}
    assert mask2.any(axis=1).all(), "mask has a fully-masked query row"
    for ib in range(NSB):
        jl = []
        for jb in range(NJB):
            sub = mask2[ib * SB:(ib + 1) * SB, jb * JB:(jb + 1) * JB]
            if not sub.any():
                continue
            sub_ops = []
            k0 = None
            for k in range(SB // JB):
                s2 = sub[k * JB:(k + 1) * JB, :]
                if s2.all():
                    sub_ops.append(("v", None))
                elif not s2.any():
                    sub_ops.append(("m", None))
                else:
                    t = np.where(s2, np.float32(1), np.float32(0)).T
                    key = t.tobytes()
                    if key not in bias_keys:
                        bias_keys[key] = len(bias_tiles)
                        bias_tiles.append(t)
                    sub_ops.append(("x", bias_keys[key]))
                if k0 is None and sub_ops[-1][0] != "m":
                    k0 = k
            jl.append((jb, k0, sub_ops))
        sched.append(jl)
    return sched, bias_tiles


def _build(sched, nbias):
    nc = bacc.Bacc()

    q8d = nc.dram_tensor("q8", [NDD, 128, NSB, 2, SB], BF16,
                         kind="ExternalInput")
    k8d = nc.dram_tensor("k8", [NDD, 128, NSB, 2, SB], BF16,
                         kind="ExternalInput")
    v8d = nc.dram_tensor("v8", [NDD, 128, NSB, 2, SB], BF16,
                         kind="ExternalInput")
    wq8d = nc.dram_tensor("wq8", [128, ND, OL], BF16, kind="ExternalInput")
    wk8d = nc.dram_tensor("wk8", [128, ND, OL], BF16, kind="ExternalInput")
    wv8d = nc.dram_tensor("wv8", [128, ND, OL], BF16, kind="ExternalInput")
    wod = nc.dram_tensor("woT", [OL, D], F32R, kind="ExternalInput")
    bqd = nc.dram_tensor("bq", [128, 2], F32, kind="ExternalInput")
    bkd = nc.dram_tensor("bk", [128, 2], F32, kind="ExternalInput")
    oned = nc.dram_tensor("ones1", [1, DK], F32R, kind="ExternalInput")
    if nbias:
        trid = nc.dram_tensor("tri", [nbias, JB, JB], BF16,
                              kind="ExternalInput")
    out = nc.dram_tensor("out", [D, S], BF16, kind="ExternalOutput")

    # head h -> (QT tile index, partition base) in the [256, S] layout
    heads = [(h // 2, DK * (h % 2)) for h in range(HPC)]

    with tile.TileContext(nc) as tc:
        with tc.tile_pool(name="consts", bufs=1) as consts:
            W8 = {n: consts.tile([128, ND, OL], BF16, name=f"W8{n}")
                  for n in "qkv"}
            WO = [consts.tile([128, D], F32R, name=f"WO{t}") for t in range(2)]
            BQ = consts.tile([128, 2], F32, name="BQ")
            BK = consts.tile([128, 2], F32, name="BK")
            TRI = [consts.tile([JB, JB], BF16, name=f"TRI{i}")
                   for i in range(nbias)]
            QT = [consts.tile([128, S], BF16, name=f"QT{t}") for t in range(2)]
            # per-head zero-padded K: head h rows at (h%2)*64, rest 0
            KP = [consts.tile([128, S], BF16, name=f"KP{h}")
                  for h in range(HPC)]
            XT = [consts.tile([128, S], F32R, name=f"XT{t}") for t in range(2)]
            ONE = consts.tile([1, DK], F32R, name="ONE")
            # V8t[u]: j-block pair u, planes t=jb%2, per-head 128-col slot
            V8t = [consts.tile([128, 2, HPC, 128], BF16, name=f"V8_{u}")
                   for u in range(NJB // 2)]

            for h in range(HPC):
                # zero the other head's contraction rows once
                z0 = DK if h % 2 == 0 else 0
                nc.vector.memset(KP[h][z0:z0 + DK, :], 0.0)

            with tc.tile_pool(name="actp", bufs=26) as actp, \
                 tc.tile_pool(name="pS", bufs=2, space="PSUM") as pS, \
                 tc.tile_pool(name="pO", bufs=2, space="PSUM") as pO, \
                 tc.tile_pool(name="pA", bufs=2, space="PSUM") as pA, \
                 tc.tile_pool(name="pP", bufs=10) as pP, \
                 tc.tile_pool(name="prr", bufs=4) as prr, \
                 tc.tile_pool(name="prc", bufs=4) as prc, \
                 tc.tile_pool(name="obuf", bufs=4) as outp:

                acts = {"q": q8d, "k": k8d, "v": v8d}
                act_tiles = {}

                def act_tile(nm, sb, dd):
                    key = (nm, sb, dd)
                    if key not in act_tiles:
                        at = actp.tile([128, 2, SB], BF16, tag="act",
                                       name="at")
                        nc.sync.dma_start(at[:], acts[nm][dd, :, sb, :, :])
                        act_tiles[key] = at
                    return act_tiles[key]

                def prefetch(nm, sb):
                    for dd in range(NDD):
                        act_tile(nm, sb, dd)

                # ---- filler units (each holds one pA psum for its span) --
                def unit_qk(nm, sb, ot):
                    wsb, bias = (W8["q"], BQ) if nm == "q" else (W8["k"], BK)
                    ps = pA.tile([128, SB], F32, tag="A", name="psA")
                    for d in range(ND):
                        at = act_tile(nm, sb, d // 2)
                        nc.tensor.matmul(
                            ps[:], wsb[:, d, ot * 128:(ot + 1) * 128],
                            at[:, d % 2, :],
                            start=(d == 0), stop=(d == ND - 1))
                    if nm == "q":
                        nc.vector.tensor_scalar_add(
                            QT[ot][:, sb * SB:(sb + 1) * SB], ps[:],
                            bias[:, ot:ot + 1])
                    else:
                        # split into the two per-head padded K tiles
                        for hh in range(2):
                            h = 2 * ot + hh
                            r0 = (h % 2) * DK
                            nc.vector.tensor_scalar_add(
                                KP[h][r0:r0 + DK, sb * SB:(sb + 1) * SB],
                                ps[r0:r0 + DK, :],
                                bias[r0:r0 + DK, ot:ot + 1])

                def unit_v(sb, st):
                    ps = pA.tile([128, OL], F32, tag="A", name="psV")
                    for d in range(ND):
                        at = act_tile("v", sb, d // 2)
                        nc.tensor.matmul(
                            ps[:], at[:, d % 2, st * 128:(st + 1) * 128],
                            W8["v"][:, d, :],
                            start=(d == 0), stop=(d == ND - 1))
                    jb = sb * 4 + st
                    u, t = jb // 2, jb % 2
                    nc.vector.tensor_copy(
                        V8t[u][:, t, :, 0:DK],
                        ps[:].rearrange("p (h c) -> p h c", c=DK))
                    nc.vector.memset(V8t[u][:, t, :, DK:VS], 1.0)

                def unit_oproj(sb, jt2):
                    for jt in (2 * jt2, 2 * jt2 + 1):
                        ps = pA.tile([128, SB], F32, tag="A", name="psO")
                        for ot in range(2):
                            nc.tensor.matmul(
                                ps[:], WO[ot][:, jt * 128:(jt + 1) * 128],
                                XT[ot][:, sb * SB:(sb + 1) * SB],
                                start=(ot == 0), stop=(ot == 1))
                        ob = outp.tile([128, SB], BF16, tag="ob", name="ob")
                        nc.vector.tensor_copy(ob[:], ps[:])
                        nc.sync.dma_start(
                            out[jt * 128:(jt + 1) * 128,
                                sb * SB:(sb + 1) * SB], ob[:])

                def proj_units(sb):
                    return ([lambda ot=ot: unit_qk("q", sb, ot)
                             for ot in range(2)]
                            + [lambda ot=ot: unit_qk("k", sb, ot)
                               for ot in range(2)]
                            + [lambda st=st: unit_v(sb, st)
                               for st in range(4)])

                def oproj_units(sb):
                    return [lambda j=j: unit_oproj(sb, j) for j in range(4)]

                # ---- lead-in -------------------------------------------
                # critical-path DMAs first: the very first Q chain needs
                # only W8q[0:4] and the first q act tile
                act_tile("q", 0, 0)
                nc.sync.dma_start(W8["q"][:, 0:4, :], wq8d[:, 0:4, :])
                act_tile("q", 0, 1)
                nc.sync.dma_start(W8["q"][:, 4:8, :], wq8d[:, 4:8, :])
                act_tile("q", 0, 2)
                act_tile("q", 0, 3)
                nc.sync.dma_start(BQ[:], bqd[:, :])
                nc.sync.dma_start(W8["k"][:], wk8d[:, :, :])
                prefetch("k", 0)
                nc.sync.dma_start(BK[:], bkd[:, :])
                nc.sync.dma_start(W8["v"][:], wv8d[:, :, :])
                prefetch("v", 0)
                for i in range(nbias):
                    nc.sync.dma_start(TRI[i][:], trid[i])
                nc.sync.dma_start(ONE[:], oned[:, :])
                # PE clock warmup: ~4us of junk matmuls with NO deps at
                # all (QT is uninitialized here; results are discarded)
                for i in range(18):
                    wps = pA.tile([128, SB], F32, tag="A", name="warm")
                    nc.tensor.matmul(
                        wps[:], QT[0][:, 0:128], QT[1][:, 0:SB],
                        start=True, stop=True)
                for un in proj_units(0):
                    un()
                for t in range(2):
                    nc.sync.dma_start(WO[t][:], wod[t * 128:(t + 1) * 128, :])

                # filler plan: which units run inside each ib's supersteps.
                # Q/K of sb3 land in ib2 (scores of ib3 need them); V of
                # sb3 is deferred to ib3's first supersteps (only PV of
                # pairs 6-7 needs it) to keep the PE fed in the long
                # ACT-heavy ib3 window.
                pu3 = proj_units(3)
                fill_by_ib = {
                    0: proj_units(1),
                    1: proj_units(2),
                    2: pu3[:4] + oproj_units(0)[:2],
                    3: pu3[4:] + oproj_units(0)[2:] + oproj_units(1)
                       + oproj_units(2),
                }

                # ---- attention: per ib, two 2-head waves -----------------
                # PV work (and each head's normalization) drains as a
                # ROLLING queue ~2 supersteps behind the scores, crossing
                # wave and ib boundaries so the PE never waits on ACT at a
                # transition. pO bufs=2 holds exactly the two in-flight
                # waves' accumulators.
                pending = []  # (h, u, kms, Pt, first, last, Ops, norm_cb)

                def emit_pv(n):
                    for h, u, kms, Pt, first, last, Ops, norm in pending[:n]:
                        for z, (t, km) in enumerate(kms):
                            c0 = km * JB
                            nc.tensor.matmul(
                                Ops[:, c0:SB],
                                V8t[u][:, t, h, 0:VS],
                                Pt[:, t, c0:SB],
                                start=(first and z == 0),
                                stop=(last and z == len(kms) - 1))
                        if last and norm is not None:
                            norm()
                    del pending[:n]

                for ib in range(NSB):
                    jl = sched[ib]
                    pairs = [jl[i:i + 2] for i in range(0, len(jl), 2)]
                    npair = len(pairs)
                    fill = fill_by_ib[ib]
                    nss = 2 * npair  # supersteps this ib
                    fpos = 0
                    if ib < NSB - 1:
                        for key in [kk for kk in act_tiles
                                    if kk[1] < ib + 1]:
                            del act_tiles[key]
                        for nm in ("q", "k", "v"):
                            prefetch(nm, ib + 1)

                    for w in range(2):
                        hw = (2 * w, 2 * w + 1)
                        Ops = {h: pO.tile([VS, SB], F32, tag="O", name="Ops")
                               for h in hw}

                        def mk_norm(h, Opsh, ib=ib):
                            hp, bp = heads[h]

                            def norm():
                                ls = prr.tile([1, SB], F32, tag="tl",
                                              name="ls")
                                nc.vector.tensor_copy(ls[:], Opsh[DK:VS, :])
                                r0 = prr.tile([1, SB], F32, tag="r0",
                                              name="r0")
                                nc.vector.reciprocal_approx_fast(r0[:],
                                                                 ls[:])
                                rr = prr.tile([1, SB], F32R, tag="r",
                                              name="rr")
                                nc.vector.tensor_copy(rr[:], r0[:])
                                Rp = pA.tile([DK, SB], F32, tag="A",
                                             name="Rp")
                                nc.tensor.matmul(Rp[:], ONE[:], rr[:],
                                                 start=True, stop=True)
                                Rc = prc.tile([DK, SB], F32, tag="rc",
                                              name="Rc")
                                nc.vector.tensor_copy(Rc[:], Rp[:])
                                nc.vector.tensor_mul(
                                    XT[hp][bp:bp + DK,
                                           ib * SB:(ib + 1) * SB],
                                    Opsh[0:DK, :], Rc[:])
                            return norm

                        for pi, pair in enumerate(pairs):
                            assert all(jb // 2 == pair[0][0] // 2
                                       for (jb, _, _) in pair)
                            if pi == 0:
                                assert pair[0][1] == 0, \
                                    "first j-block must be visible from k=0"
                            for h in hw:
                                hp, bp = heads[h]
                                Sp = pS.tile([128, 2 * SB], F32, tag="S",
                                             name="Sp")
                                Sp3 = Sp[:].rearrange("p (z i) -> p z i",
                                                      z=2)
                                kmin = min(k0 for (_, k0, _) in pair)
                                for z, (jb, k0, sub_ops) in enumerate(pair):
                                    c0 = k0 * JB
                                    nc.tensor.matmul(
                                        Sp[:, z * SB + c0:(z + 1) * SB],
                                        KP[h][:, jb * JB:(jb + 1) * JB],
                                        QT[hp][:, ib * SB + c0:
                                               (ib + 1) * SB],
                                        start=True, stop=True)
                                Pt = pP.tile([128, 2, SB], BF16, tag="P",
                                             name="Pt")
                                nc.scalar.activation(
                                    Pt[:, :, kmin * JB:SB],
                                    Sp3[:, :, kmin * JB:SB], EXP,
                                    scale=0.125)
                                kms = []
                                for z, (jb, k0, sub_ops) in enumerate(pair):
                                    for k, (stt, bidx) in enumerate(sub_ops):
                                        if stt == "m" and k >= k0:
                                            nc.vector.memset(
                                                Pt[:, z,
                                                   k * JB:(k + 1) * JB],
                                                0.0)
                                        elif stt == "x":
                                            nc.gpsimd.tensor_mul(
                                                Pt[:, z,
                                                   k * JB:(k + 1) * JB],
                                                Pt[:, z,
                                                   k * JB:(k + 1) * JB],
                                                TRI[bidx][:])
                                    kms.append(
                                        (jb % 2,
                                         0 if pi == 0 and z == 0 else k0))
                                last = pi == npair - 1
                                pending.append(
                                    (h, pair[0][0] // 2, kms, Pt, pi == 0,
                                     last, Ops[h],
                                     mk_norm(h, Ops[h]) if last else None))
                            if len(pending) > 4:
                                emit_pv(2)
                            # filler: spread this ib's units over supersteps
                            # (front-loaded 3/4 in the last ib)
                            si = w * npair + pi
                            eff = (nss if ib < NSB - 1
                                   else max(1, (3 * nss) // 4))
                            emitted = False
                            while (fpos < len(fill)
                                   and fpos * eff <= (si + 1) * len(fill)):
                                fill[fpos]()
                                fpos += 1
                                emitted = True
                            if ib == NSB - 1 and fpos >= len(fill) \
                                    and not emitted and len(pending) <= 4:
                                # keep the PE clock hot through the
                                # ACT-bound stretch
                                wps = pA.tile([128, SB], F32, tag="A",
                                              name="warm2")
                                nc.tensor.matmul(
                                    wps[:], QT[0][:, 0:128], QT[1][:, 0:SB],
                                    start=True, stop=True)
                    while fpos < len(fill):
                        fill[fpos]()
                        fpos += 1
                # drain the remaining PV/normalization work
                while pending:
                    emit_pv(2)

                # ---- tail: out-projection for sb3 ------------------------
                for un in oproj_units(NSB - 1):
                    un()
    nc.finalize()
    return nc


def kernel(q, k, v, mask, wq, bq, wk, bk, wv, bv, wo, bo):
    global LAST_RUN
    q, k, v = (np.asarray(x, np.float32) for x in (q, k, v))
    wq, bq, wk, bk = (np.asarray(x, np.float32) for x in (wq, bq, wk, bk))
    wv, bv, wo, bo = (np.asarray(x, np.float32) for x in (wv, bv, wo, bo))
    mask2 = np.asarray(mask)[0, 0] != 0

    sched, bias_tiles = _classify_mask(mask2)
    nbias = len(bias_tiles)
    trib = np.stack(bias_tiles).astype(BF) if nbias else None

    bo_eff = (bo + wo @ bv).astype(np.float32)

    def act_layout(x):
        # [S, D] -> bf16 [NDD, 128, NSB, 2, SB] with paired d-block planes
        a8 = np.ascontiguousarray(x.T).astype(BF)          # [D, S]
        a8 = a8.reshape(NDD, 2, 128, NSB, SB)
        return np.ascontiguousarray(a8.transpose(0, 2, 3, 1, 4))

    q8s = [act_layout(q[b]) for b in range(B)]
    k8s = [act_layout(k[b]) for b in range(B)]
    v8s = [act_layout(v[b]) for b in range(B)]

    def w_layout(w):
        # rows [OL, D] slice -> bf16 [128, ND, OL]
        w8 = np.ascontiguousarray(w.T).astype(BF)          # [D, OL]
        return np.ascontiguousarray(
            w8.reshape(ND, 128, OL).transpose(1, 0, 2))

    wq8s, wk8s, wv8s, wos, bqs, bks = [], [], [], [], [], []
    for g in range(GROUPS):
        rows = slice(g * OL, (g + 1) * OL)
        wq8s.append(w_layout(wq[rows]))
        wk8s.append(w_layout(wk[rows]))
        wv8s.append(w_layout(wv[rows]))
        wos.append(_round_f32r(wo[:, rows].T))
        bqs.append(np.ascontiguousarray(
            bq[rows].reshape(2, 128).T.astype(np.float32)))
        bks.append(np.ascontiguousarray(
            bk[rows].reshape(2, 128).T.astype(np.float32)))

    in_maps = []
    for c in range(NCORES):
        b, g = c // GROUPS, c % GROUPS
        m = {
            "q8": q8s[b], "k8": k8s[b], "v8": v8s[b],
            "wq8": wq8s[g], "wk8": wk8s[g], "wv8": wv8s[g],
            "woT": wos[g], "bq": bqs[g], "bk": bks[g],
            "ones1": np.ones((1, DK), np.float32),
        }
        if nbias:
            m["tri"] = trib
        in_maps.append(m)

    nc = _build(sched, nbias)
    res = run_bass_kernel_spmd(nc, in_maps, core_ids=list(range(NCORES)))
    LAST_RUN = res
    if res.exec_time_ns is not None:
        print(f"HW exec time: {res.exec_time_ns} ns")

    outp = np.zeros((B, S, D), np.float32)
    for c in range(NCORES):
        b = c // GROUPS
        outp[b] += np.asarray(res.results[c]["out"]).astype(np.float32).T
    outp += bo_eff
    return outp


# revision 36
# speedup vs baseline: 1.0369x; 1.0369x over previous
"""Multi-head attention block (B=2, S=2048, D=1024, H=16) on 8 trn2 cores.

Sharding: core c = (batch b = c//4, head-group g = c%4); each core computes
4 heads of one batch (Megatron column-shard of wq/wk/wv, row-shard of wo,
combined with data-parallel over batch). Host sums the 4 partial outputs
per batch and adds the (folded) bias.

v4: all-bf16, single continuously-fed PE stream.

Numerics: the attention output is a near-uniform average of ~2k value
rows, so per-element quantization error lands ~1:1 on the final output;
fp8 e4m3 (~3%/stage) blows the 2e-2 budget while bf16 (~0.1%) is free.
All matmuls run bf16 (1 col/cycle) except the f32r out-projection.

Performance: the TRN2 PE clock ramps — 2.4 GHz only after ~3us of
continuous execution, 1.2 GHz after any stall. The whole kernel is
emitted as one interleaved PE stream that never idles:
  - scores use per-head zero-PADDED K tiles [128, S] so the contraction
    is full 128 partitions (64-partition bf16 matmuls stream ~2x slower)
    with the 2-head QT tile as moving operand (the other head's rows are
    zeroed in K, so its contribution vanishes)
  - causal masking is a post-exp multiply by a shared 0/1 triangle on
    GPSIMD (off the PE/ACT critical path); diagonal blocks stream only
    the visible column suffix
  - exp on ACT: psum f32 [128,2x512] (one j-block PAIR per head) ->
    bf16 P, scale=1/8
  - PV per j-block: stationary V [128,65] (ones column -> row-sums l)
  - projection chains (Q/K/V per sb) and out-projection tiles are
    emitted as FILLER units inside the ACT-gated attention window, so
    the PE always has queued work and stays at full clock
PSUM: S-pair tiles 2x2 banks, O (PV accum) 2, filler chains 2 = 8.
Output [D, S] bf16 per core; host sums 4 partials per batch in f32 and
adds bo_eff = bo + wo@bv.
"""

import numpy as np
import ml_dtypes

import concourse.bass as bass
import concourse.mybir as mybir
import concourse.tile as tile
from concourse import bacc
from concourse.bass_utils import run_bass_kernel_spmd

B, S, D, H = 2, 2048, 1024, 16
DK = D // H                  # 64
NCORES = 8
GROUPS = NCORES // B         # 4 head-groups
HPC = H // GROUPS            # 4 heads per core
OL = HPC * DK                # 256 local features
SB = 512                     # query-block (i) width
JB = 128                     # key-block (j) width
NSB = S // SB                # 4
NJB = S // JB                # 16
ND = D // 128                # 8 contraction blocks
NDD = ND // 2                # act DMA pairing
VS = DK + 1                  # V columns per head incl. ones column (65)

F32 = mybir.dt.float32
F32R = mybir.dt.float32r
BF16 = mybir.dt.bfloat16
EXP = mybir.ActivationFunctionType.Exp

BF = ml_dtypes.bfloat16

LAST_RUN = None  # stash of BassKernelResults for test harness inspection


def _round_f32r(a):
    """Round an f32 array to the f32r grid (top-20-bit float)."""
    a = np.ascontiguousarray(a, np.float32)
    u = a.view(np.uint32)
    u = (u + 0x7FF + ((u >> 12) & 1)) & np.uint32(0xFFFFF000)
    return u.view(np.float32)


def _classify_mask(mask2):
    """Block schedule from the boolean mask [S, S] (True = visible).

    sched[ib] = list of (jb, k0, sub_ops); k0 = first visible 128-i-sub;
    sub_ops[k] in ('v', None) | ('m', None) | ('x', bias_idx).
    bias_tiles: deduplicated [n, JB, JB] multiplicative masks
    (transposed [j, i]), 1 visible / 0 masked.
    """
    sched = []
    bias_tiles = []
    bias_keys = {}
    assert mask2.any(axis=1).all(), "mask has a fully-masked query row"
    for ib in range(NSB):
        jl = []
        for jb in range(NJB):
            sub = mask2[ib * SB:(ib + 1) * SB, jb * JB:(jb + 1) * JB]
            if not sub.any():
                continue
            sub_ops = []
            k0 = None
            for k in range(SB // JB):
                s2 = sub[k * JB:(k + 1) * JB, :]
                if s2.all():
                    sub_ops.append(("v", None))
                elif not s2.any():
                    sub_ops.append(("m", None))
                else:
                    t = np.where(s2, np.float32(1), np.float32(0)).T
                    key = t.tobytes()
                    if key not in bias_keys:
                        bias_keys[key] = len(bias_tiles)
                        bias_tiles.append(t)
                    sub_ops.append(("x", bias_keys[key]))
                if k0 is None and sub_ops[-1][0] != "m":
                    k0 = k
            jl.append((jb, k0, sub_ops))
        sched.append(jl)
    return sched, bias_tiles


def _build(sched, nbias):
    nc = bacc.Bacc()

    q8d = nc.dram_tensor("q8", [NDD, 128, NSB, 2, SB], BF16,
                         kind="ExternalInput")
    k8d = nc.dram_tensor("k8", [NDD, 128, NSB, 2, SB], BF16,
                         kind="ExternalInput")
    v8d = nc.dram_tensor("v8", [NDD, 128, NSB, 2, SB], BF16,
                         kind="ExternalInput")
    wq8d = nc.dram_tensor("wq8", [128, ND, OL], BF16, kind="ExternalInput")
    wk8d = nc.dram_tensor("wk8", [128, ND, OL], BF16, kind="ExternalInput")
    wv8d = nc.dram_tensor("wv8", [128, ND, OL], BF16, kind="ExternalInput")
    wod = nc.dram_tensor("woT", [OL, D], F32R, kind="ExternalInput")
    bqd = nc.dram_tensor("bq", [128, 2], F32, kind="ExternalInput")
    bkd = nc.dram_tensor("bk", [128, 2], F32, kind="ExternalInput")
    oned = nc.dram_tensor("ones1", [1, DK], F32R, kind="ExternalInput")
    if nbias:
        trid = nc.dram_tensor("tri", [nbias, JB, JB], BF16,
                              kind="ExternalInput")
    out = nc.dram_tensor("out", [D, S], BF16, kind="ExternalOutput")

    # head h -> (QT tile index, partition base) in the [256, S] layout
    heads = [(h // 2, DK * (h % 2)) for h in range(HPC)]

    with tile.TileContext(nc) as tc:
        with tc.tile_pool(name="consts", bufs=1) as consts:
            W8 = {n: consts.tile([128, ND, OL], BF16, name=f"W8{n}")
                  for n in "qkv"}
            WO = [consts.tile([128, D], F32R, name=f"WO{t}") for t in range(2)]
            BQ = consts.tile([128, 2], F32, name="BQ")
            BK = consts.tile([128, 2], F32, name="BK")
            TRI = [consts.tile([JB, JB], BF16, name=f"TRI{i}")
                   for i in range(nbias)]
            QT = [consts.tile([128, S], BF16, name=f"QT{t}") for t in range(2)]
            # per-head zero-padded K: head h rows at (h%2)*64, rest 0
            KP = [consts.tile([128, S], BF16, name=f"KP{h}")
                  for h in range(HPC)]
            XT = [consts.tile([128, S], F32R, name=f"XT{t}") for t in range(2)]
            ONE = consts.tile([1, DK], F32R, name="ONE")
            # V8t[u]: j-block pair u, planes t=jb%2, per-head 128-col slot
            V8t = [consts.tile([128, 2, HPC, 128], BF16, name=f"V8_{u}")
                   for u in range(NJB // 2)]

            for h in range(HPC):
                # zero the other head's contraction rows once
                z0 = DK if h % 2 == 0 else 0
                nc.vector.memset(KP[h][z0:z0 + DK, :], 0.0)

            with tc.tile_pool(name="actp", bufs=26) as actp, \
                 tc.tile_pool(name="pS", bufs=2, space="PSUM") as pS, \
                 tc.tile_pool(name="pO", bufs=2, space="PSUM") as pO, \
                 tc.tile_pool(name="pA", bufs=2, space="PSUM") as pA, \
                 tc.tile_pool(name="pP", bufs=8) as pP, \
                 tc.tile_pool(name="prr", bufs=4) as prr, \
                 tc.tile_pool(name="prc", bufs=4) as prc, \
                 tc.tile_pool(name="obuf", bufs=4) as outp:

                acts = {"q": q8d, "k": k8d, "v": v8d}
                act_tiles = {}

                def act_tile(nm, sb, dd):
                    key = (nm, sb, dd)
                    if key not in act_tiles:
                        at = actp.tile([128, 2, SB], BF16, tag="act",
                                       name="at")
                        nc.sync.dma_start(at[:], acts[nm][dd, :, sb, :, :])
                        act_tiles[key] = at
                    return act_tiles[key]

                def prefetch(nm, sb):
                    for dd in range(NDD):
                        act_tile(nm, sb, dd)

                # ---- filler units (each holds one pA psum for its span) --
                def unit_qk(nm, sb, ot):
                    wsb, bias = (W8["q"], BQ) if nm == "q" else (W8["k"], BK)
                    ps = pA.tile([128, SB], F32, tag="A", name="psA")
                    for d in range(ND):
                        at = act_tile(nm, sb, d // 2)
                        nc.tensor.matmul(
                            ps[:], wsb[:, d, ot * 128:(ot + 1) * 128],
                            at[:, d % 2, :],
                            start=(d == 0), stop=(d == ND - 1))
                    if nm == "q":
                        nc.vector.tensor_scalar_add(
                            QT[ot][:, sb * SB:(sb + 1) * SB], ps[:],
                            bias[:, ot:ot + 1])
                    else:
                        # split into the two per-head padded K tiles
                        for hh in range(2):
                            h = 2 * ot + hh
                            r0 = (h % 2) * DK
                            nc.vector.tensor_scalar_add(
                                KP[h][r0:r0 + DK, sb * SB:(sb + 1) * SB],
                                ps[r0:r0 + DK, :],
                                bias[r0:r0 + DK, ot:ot + 1])

                def unit_v(sb, st):
                    ps = pA.tile([128, OL], F32, tag="A", name="psV")
                    for d in range(ND):
                        at = act_tile("v", sb, d // 2)
                        nc.tensor.matmul(
                            ps[:], at[:, d % 2, st * 128:(st + 1) * 128],
                            W8["v"][:, d, :],
                            start=(d == 0), stop=(d == ND - 1))
                    jb = sb * 4 + st
                    u, t = jb // 2, jb % 2
                    nc.vector.tensor_copy(
                        V8t[u][:, t, :, 0:DK],
                        ps[:].rearrange("p (h c) -> p h c", c=DK))
                    nc.vector.memset(V8t[u][:, t, :, DK:VS], 1.0)

                def unit_oproj(sb, jt2):
                    for jt in (2 * jt2, 2 * jt2 + 1):
                        ps = pA.tile([128, SB], F32, tag="A", name="psO")
                        for ot in range(2):
                            nc.tensor.matmul(
                                ps[:], WO[ot][:, jt * 128:(jt + 1) * 128],
                                XT[ot][:, sb * SB:(sb + 1) * SB],
                                start=(ot == 0), stop=(ot == 1))
                        ob = outp.tile([128, SB], BF16, tag="ob", name="ob")
                        nc.vector.tensor_copy(ob[:], ps[:])
                        nc.sync.dma_start(
                            out[jt * 128:(jt + 1) * 128,
                                sb * SB:(sb + 1) * SB], ob[:])

                def proj_units(sb):
                    return ([lambda ot=ot: unit_qk("q", sb, ot)
                             for ot in range(2)]
                            + [lambda ot=ot: unit_qk("k", sb, ot)
                               for ot in range(2)]
                            + [lambda st=st: unit_v(sb, st)
                               for st in range(4)])

                def oproj_units(sb):
                    return [lambda j=j: unit_oproj(sb, j) for j in range(4)]

                # ---- lead-in -------------------------------------------
                # critical-path DMAs first: the very first Q chain needs
                # only W8q[0:4] and the first q act tile
                nc.sync.dma_start(W8["q"][:, 0:4, :], wq8d[:, 0:4, :])
                act_tile("q", 0, 0)
                act_tile("q", 0, 1)
                nc.sync.dma_start(W8["q"][:, 4:8, :], wq8d[:, 4:8, :])
                act_tile("q", 0, 2)
                act_tile("q", 0, 3)
                nc.sync.dma_start(BQ[:], bqd[:, :])
                nc.sync.dma_start(W8["k"][:], wk8d[:, :, :])
                prefetch("k", 0)
                nc.sync.dma_start(BK[:], bkd[:, :])
                nc.sync.dma_start(W8["v"][:], wv8d[:, :, :])
                prefetch("v", 0)
                for i in range(nbias):
                    nc.sync.dma_start(TRI[i][:], trid[i])
                nc.sync.dma_start(ONE[:], oned[:, :])
                # PE clock warmup: ~4us of junk matmuls with NO deps at
                # all (QT is uninitialized here; results are discarded)
                for i in range(18):
                    wps = pA.tile([128, SB], F32, tag="A", name="warm")
                    nc.tensor.matmul(
                        wps[:], QT[0][:, 0:128], QT[1][:, 0:SB],
                        start=True, stop=True)
                for un in proj_units(0):
                    un()
                for t in range(2):
                    nc.sync.dma_start(WO[t][:], wod[t * 128:(t + 1) * 128, :])

                # filler plan: which units run inside each ib's supersteps.
                # Q/K of sb3 land in ib2 (scores of ib3 need them); V of
                # sb3 is deferred to ib3's first supersteps (only PV of
                # pairs 6-7 needs it) to keep the PE fed in the long
                # ACT-heavy ib3 window.
                pu3 = proj_units(3)
                fill_by_ib = {
                    0: proj_units(1),
                    1: proj_units(2),
                    2: pu3[:4] + oproj_units(0)[:2],
                    3: pu3[4:] + oproj_units(0)[2:] + oproj_units(1)
                       + oproj_units(2),
                }

                # ---- attention: per ib, two 2-head waves -----------------
                # PV work (and each head's normalization) drains as a
                # ROLLING queue ~2 supersteps behind the scores, crossing
                # wave and ib boundaries so the PE never waits on ACT at a
                # transition. pO bufs=2 holds exactly the two in-flight
                # waves' accumulators.
                pending = []  # (h, u, kms, Pt, first, last, Ops, norm_cb)

                def emit_pv(n):
                    for h, u, kms, Pt, first, last, Ops, norm in pending[:n]:
                        for z, (t, km) in enumerate(kms):
                            c0 = km * JB
                            nc.tensor.matmul(
                                Ops[:, c0:SB],
                                V8t[u][:, t, h, 0:VS],
                                Pt[:, t, c0:SB],
                                start=(first and z == 0),
                                stop=(last and z == len(kms) - 1))
                        if last and norm is not None:
                            norm()
                    del pending[:n]

                for ib in range(NSB):
                    jl = sched[ib]
                    pairs = [jl[i:i + 2] for i in range(0, len(jl), 2)]
                    npair = len(pairs)
                    fill = fill_by_ib[ib]
                    nss = 2 * npair  # supersteps this ib
                    fpos = 0
                    if ib < NSB - 1:
                        for key in [kk for kk in act_tiles
                                    if kk[1] < ib + 1]:
                            del act_tiles[key]
                        for nm in ("q", "k", "v"):
                            prefetch(nm, ib + 1)

                    for w in range(2):
                        hw = (2 * w, 2 * w + 1)
                        Ops = {h: pO.tile([VS, SB], F32, tag="O", name="Ops")
                               for h in hw}

                        def mk_norm(h, Opsh, ib=ib):
                            hp, bp = heads[h]

                            def norm():
                                ls = prr.tile([1, SB], F32, tag="tl",
                                              name="ls")
                                nc.vector.tensor_copy(ls[:], Opsh[DK:VS, :])
                                r0 = prr.tile([1, SB], F32, tag="r0",
                                              name="r0")
                                nc.vector.reciprocal_approx_fast(r0[:],
                                                                 ls[:])
                                rr = prr.tile([1, SB], F32R, tag="r",
                                              name="rr")
                                nc.vector.tensor_copy(rr[:], r0[:])
                                Rp = pA.tile([DK, SB], F32, tag="A",
                                             name="Rp")
                                nc.tensor.matmul(Rp[:], ONE[:], rr[:],
                                                 start=True, stop=True)
                                Rc = prc.tile([DK, SB], F32, tag="rc",
                                              name="Rc")
                                nc.vector.tensor_copy(Rc[:], Rp[:])
                                nc.vector.tensor_mul(
                                    XT[hp][bp:bp + DK,
                                           ib * SB:(ib + 1) * SB],
                                    Opsh[0:DK, :], Rc[:])
                            return norm

                        for pi, pair in enumerate(pairs):
                            assert all(jb // 2 == pair[0][0] // 2
                                       for (jb, _, _) in pair)
                            if pi == 0:
                                assert pair[0][1] == 0, \
                                    "first j-block must be visible from k=0"
                            for h in hw:
                                hp, bp = heads[h]
                                Sp = pS.tile([128, 2 * SB], F32, tag="S",
                                             name="Sp")
                                Sp3 = Sp[:].rearrange("p (z i) -> p z i",
                                                      z=2)
                                kmin = min(k0 for (_, k0, _) in pair)
                                for z, (jb, k0, sub_ops) in enumerate(pair):
                                    c0 = k0 * JB
                                    nc.tensor.matmul(
                                        Sp[:, z * SB + c0:(z + 1) * SB],
                                        KP[h][:, jb * JB:(jb + 1) * JB],
                                        QT[hp][:, ib * SB + c0:
                                               (ib + 1) * SB],
                                        start=True, stop=True)
                                Pt = pP.tile([128, 2, SB], BF16, tag="P",
                                             name="Pt")
                                nc.scalar.activation(
                                    Pt[:, :, kmin * JB:SB],
                                    Sp3[:, :, kmin * JB:SB], EXP,
                                    scale=0.125)
                                kms = []
                                for z, (jb, k0, sub_ops) in enumerate(pair):
                                    for k, (stt, bidx) in enumerate(sub_ops):
                                        if stt == "m" and k >= k0:
                                            nc.vector.memset(
                                                Pt[:, z,
                                                   k * JB:(k + 1) * JB],
                                                0.0)
                                        elif stt == "x":
                                            nc.gpsimd.tensor_mul(
                                                Pt[:, z,
                                                   k * JB:(k + 1) * JB],
                                                Pt[:, z,
                                                   k * JB:(k + 1) * JB],
                                                TRI[bidx][:])
                                    kms.append(
                                        (jb % 2,
                                         0 if pi == 0 and z == 0 else k0))
                                last = pi == npair - 1
                                pending.append(
                                    (h, pair[0][0] // 2, kms, Pt, pi == 0,
                                     last, Ops[h],
                                     mk_norm(h, Ops[h]) if last else None))
                            if len(pending) > 4:
                                emit_pv(2)
                            # filler: spread this ib's units over supersteps
                            # (front-loaded 3/4 in the last ib)
                            si = w * npair + pi
                            eff = (nss if ib < NSB - 1
                                   else max(1, (3 * nss) // 4))
                            emitted = False
                            while (fpos < len(fill)
                                   and fpos * eff <= (si + 1) * len(fill)):
                                fill[fpos]()
                                fpos += 1
                                emitted = True
                            if ib == NSB - 1 and fpos >= len(fill) \
                                    and not emitted and len(pending) <= 4:
                                # keep the PE clock hot through the
                                # ACT-bound stretch
                                wps = pA.tile([128, SB], F32, tag="A",
                                              name="warm2")
                                nc.tensor.matmul(
                                    wps[:], QT[0][:, 0:128], QT[1][:, 0:SB],
                                    start=True, stop=True)
                    while fpos < len(fill):
                        fill[fpos]()
                        fpos += 1
                # drain the remaining PV/normalization work
                while pending:
                    emit_pv(2)

                # ---- tail: out-projection for sb3 ------------------------
                for un in oproj_units(NSB - 1):
                    un()
    nc.finalize()
    return nc


def kernel(q, k, v, mask, wq, bq, wk, bk, wv, bv, wo, bo):
    global LAST_RUN
    q, k, v = (np.asarray(x, np.float32) for x in (q, k, v))
    wq, bq, wk, bk = (np.asarray(x, np.float32) for x in (wq, bq, wk, bk))
    wv, bv, wo, bo = (np.asarray(x, np.float32) for x in (wv, bv, wo, bo))
    mask2 = np.asarray(mask)[0, 0] != 0

    sched, bias_tiles = _classify_mask(mask2)
    nbias = len(bias_tiles)
    trib = np.stack(bias_tiles).astype(BF) if nbias else None

    bo_eff = (bo + wo @ bv).astype(np.float32)

    def act_layout(x):
        # [S, D] -> bf16 [NDD, 128, NSB, 2, SB] with paired d-block planes
        a8 = np.ascontiguousarray(x.T).astype(BF)          # [D, S]
        a8 = a8.reshape(NDD, 2, 128, NSB, SB)
        return np.ascontiguousarray(a8.transpose(0, 2, 3, 1, 4))

    q8s = [act_layout(q[b]) for b in range(B)]
    k8s = [act_layout(k[b]) for b in range(B)]
    v8s = [act_layout(v[b]) for b in range(B)]

    def w_layout(w):
        # rows [OL, D] slice -> bf16 [128, ND, OL]
        w8 = np.ascontiguousarray(w.T).astype(BF)          # [D, OL]
        return np.ascontiguousarray(
            w8.reshape(ND, 128, OL).transpose(1, 0, 2))

    wq8s, wk8s, wv8s, wos, bqs, bks = [], [], [], [], [], []
    for g in range(GROUPS):
        rows = slice(g * OL, (g + 1) * OL)
        wq8s.append(w_layout(wq[rows]))
        wk8s.append(w_layout(wk[rows]))
        wv8s.append(w_layout(wv[rows]))
        wos.append(_round_f32r(wo[:, rows].T))
        bqs.append(np.ascontiguousarray(
            bq[rows].reshape(2, 128).T.astype(np.float32)))
        bks.append(np.ascontiguousarray(
            bk[rows].reshape(2, 128).T.astype(np.float32)))

    in_maps = []
    for c in range(NCORES):
        b, g = c // GROUPS, c % GROUPS
        m = {
            "q8": q8s[b], "k8": k8s[b], "v8": v8s[b],
            "wq8": wq8s[g], "wk8": wk8s[g], "wv8": wv8s[g],
            "woT": wos[g], "bq": bqs[g], "bk": bks[g],
            "ones1": np.ones((1, DK), np.float32),
        }
        if nbias:
            m["tri"] = trib
        in_maps.append(m)

    nc = _build(sched, nbias)
    res = run_bass_kernel_spmd(nc, in_maps, core_ids=list(range(NCORES)))
    LAST_RUN = res
    if res.exec_time_ns is not None:
        print(f"HW exec time: {res.exec_time_ns} ns")

    outp = np.zeros((B, S, D), np.float32)
    for c in range(NCORES):
        b = c // GROUPS
        outp[b] += np.asarray(res.results[c]["out"]).astype(np.float32).T
    outp += bo_eff
    return outp


# revision 37
# speedup vs baseline: 1.0374x; 1.0004x over previous
"""Multi-head attention block (B=2, S=2048, D=1024, H=16) on 8 trn2 cores.

Sharding: core c = (batch b = c//4, head-group g = c%4); each core computes
4 heads of one batch (Megatron column-shard of wq/wk/wv, row-shard of wo,
combined with data-parallel over batch). Host sums the 4 partial outputs
per batch and adds the (folded) bias.

v4: all-bf16, single continuously-fed PE stream.

Numerics: the attention output is a near-uniform average of ~2k value
rows, so per-element quantization error lands ~1:1 on the final output;
fp8 e4m3 (~3%/stage) blows the 2e-2 budget while bf16 (~0.1%) is free.
All matmuls run bf16 (1 col/cycle) except the f32r out-projection.

Performance: the TRN2 PE clock ramps — 2.4 GHz only after ~3us of
continuous execution, 1.2 GHz after any stall. The whole kernel is
emitted as one interleaved PE stream that never idles:
  - scores use per-head zero-PADDED K tiles [128, S] so the contraction
    is full 128 partitions (64-partition bf16 matmuls stream ~2x slower)
    with the 2-head QT tile as moving operand (the other head's rows are
    zeroed in K, so its contribution vanishes)
  - causal masking is a post-exp multiply by a shared 0/1 triangle on
    GPSIMD (off the PE/ACT critical path); diagonal blocks stream only
    the visible column suffix
  - exp on ACT: psum f32 [128,2x512] (one j-block PAIR per head) ->
    bf16 P, scale=1/8
  - PV per j-block: stationary V [128,65] (ones column -> row-sums l)
  - projection chains (Q/K/V per sb) and out-projection tiles are
    emitted as FILLER units inside the ACT-gated attention window, so
    the PE always has queued work and stays at full clock
PSUM: S-pair tiles 2x2 banks, O (PV accum) 2, filler chains 2 = 8.
Output [D, S] bf16 per core; host sums 4 partials per batch in f32 and
adds bo_eff = bo + wo@bv.
"""

import numpy as np
import ml_dtypes

import concourse.bass as bass
import concourse.mybir as mybir
import concourse.tile as tile
from concourse import bacc
from concourse.bass_utils import run_bass_kernel_spmd

B, S, D, H = 2, 2048, 1024, 16
DK = D // H                  # 64
NCORES = 8
GROUPS = NCORES // B         # 4 head-groups
HPC = H // GROUPS            # 4 heads per core
OL = HPC * DK                # 256 local features
SB = 512                     # query-block (i) width
JB = 128                     # key-block (j) width
NSB = S // SB                # 4
NJB = S // JB                # 16
ND = D // 128                # 8 contraction blocks
NDD = ND // 2                # act DMA pairing
VS = DK + 1                  # V columns per head incl. ones column (65)

F32 = mybir.dt.float32
F32R = mybir.dt.float32r
BF16 = mybir.dt.bfloat16
EXP = mybir.ActivationFunctionType.Exp

BF = ml_dtypes.bfloat16

LAST_RUN = None  # stash of BassKernelResults for test harness inspection


def _round_f32r(a):
    """Round an f32 array to the f32r grid (top-20-bit float)."""
    a = np.ascontiguousarray(a, np.float32)
    u = a.view(np.uint32)
    u = (u + 0x7FF + ((u >> 12) & 1)) & np.uint32(0xFFFFF000)
    return u.view(np.float32)


def _classify_mask(mask2):
    """Block schedule from the boolean mask [S, S] (True = visible).

    sched[ib] = list of (jb, k0, sub_ops); k0 = first visible 128-i-sub;
    sub_ops[k] in ('v', None) | ('m', None) | ('x', bias_idx).
    bias_tiles: deduplicated [n, JB, JB] multiplicative masks
    (transposed [j, i]), 1 visible / 0 masked.
    """
    sched = []
    bias_tiles = []
    bias_keys = {}
    assert mask2.any(axis=1).all(), "mask has a fully-masked query row"
    for ib in range(NSB):
        jl = []
        for jb in range(NJB):
            sub = mask2[ib * SB:(ib + 1) * SB, jb * JB:(jb + 1) * JB]
            if not sub.any():
                continue
            sub_ops = []
            k0 = None
            for k in range(SB // JB):
                s2 = sub[k * JB:(k + 1) * JB, :]
                if s2.all():
                    sub_ops.append(("v", None))
                elif not s2.any():
                    sub_ops.append(("m", None))
                else:
                    t = np.where(s2, np.float32(1), np.float32(0)).T
                    key = t.tobytes()
                    if key not in bias_keys:
                        bias_keys[key] = len(bias_tiles)
                        bias_tiles.append(t)
                    sub_ops.append(("x", bias_keys[key]))
                if k0 is None and sub_ops[-1][0] != "m":
                    k0 = k
            jl.append((jb, k0, sub_ops))
        sched.append(jl)
    return sched, bias_tiles


def _build(sched, nbias):
    nc = bacc.Bacc()

    q8d = nc.dram_tensor("q8", [NDD, 128, NSB, 2, SB], BF16,
                         kind="ExternalInput")
    k8d = nc.dram_tensor("k8", [NDD, 128, NSB, 2, SB], BF16,
                         kind="ExternalInput")
    v8d = nc.dram_tensor("v8", [NDD, 128, NSB, 2, SB], BF16,
                         kind="ExternalInput")
    wq8d = nc.dram_tensor("wq8", [128, ND, OL], BF16, kind="ExternalInput")
    wk8d = nc.dram_tensor("wk8", [128, ND, OL], BF16, kind="ExternalInput")
    wv8d = nc.dram_tensor("wv8", [128, ND, OL], BF16, kind="ExternalInput")
    wod = nc.dram_tensor("woT", [OL, D], F32R, kind="ExternalInput")
    bqd = nc.dram_tensor("bq", [128, 2], F32, kind="ExternalInput")
    bkd = nc.dram_tensor("bk", [128, 2], F32, kind="ExternalInput")
    oned = nc.dram_tensor("ones1", [1, DK], F32R, kind="ExternalInput")
    if nbias:
        trid = nc.dram_tensor("tri", [nbias, JB, JB], BF16,
                              kind="ExternalInput")
    out = nc.dram_tensor("out", [D, S], BF16, kind="ExternalOutput")

    # head h -> (QT tile index, partition base) in the [256, S] layout
    heads = [(h // 2, DK * (h % 2)) for h in range(HPC)]

    with tile.TileContext(nc) as tc:
        with tc.tile_pool(name="consts", bufs=1) as consts:
            W8 = {n: consts.tile([128, ND, OL], BF16, name=f"W8{n}")
                  for n in "qkv"}
            WO = [consts.tile([128, D], F32R, name=f"WO{t}") for t in range(2)]
            BQ = consts.tile([128, 2], F32, name="BQ")
            BK = consts.tile([128, 2], F32, name="BK")
            TRI = [consts.tile([JB, JB], BF16, name=f"TRI{i}")
                   for i in range(nbias)]
            QT = [consts.tile([128, S], BF16, name=f"QT{t}") for t in range(2)]
            # per-head zero-padded K: head h rows at (h%2)*64, rest 0
            KP = [consts.tile([128, S], BF16, name=f"KP{h}")
                  for h in range(HPC)]
            XT = [consts.tile([128, S], F32R, name=f"XT{t}") for t in range(2)]
            ONE = consts.tile([1, DK], F32R, name="ONE")
            # V8t[u]: j-block pair u, planes t=jb%2, per-head 128-col slot
            V8t = [consts.tile([128, 2, HPC, 128], BF16, name=f"V8_{u}")
                   for u in range(NJB // 2)]

            for h in range(HPC):
                # zero the other head's contraction rows once
                z0 = DK if h % 2 == 0 else 0
                nc.vector.memset(KP[h][z0:z0 + DK, :], 0.0)

            with tc.tile_pool(name="actp", bufs=26) as actp, \
                 tc.tile_pool(name="pS", bufs=2, space="PSUM") as pS, \
                 tc.tile_pool(name="pO", bufs=2, space="PSUM") as pO, \
                 tc.tile_pool(name="pA", bufs=2, space="PSUM") as pA, \
                 tc.tile_pool(name="pP", bufs=8) as pP, \
                 tc.tile_pool(name="prr", bufs=4) as prr, \
                 tc.tile_pool(name="prc", bufs=4) as prc, \
                 tc.tile_pool(name="obuf", bufs=4) as outp:

                acts = {"q": q8d, "k": k8d, "v": v8d}
                act_tiles = {}

                def act_tile(nm, sb, dd):
                    key = (nm, sb, dd)
                    if key not in act_tiles:
                        at = actp.tile([128, 2, SB], BF16, tag="act",
                                       name="at")
                        nc.sync.dma_start(at[:], acts[nm][dd, :, sb, :, :])
                        act_tiles[key] = at
                    return act_tiles[key]

                def prefetch(nm, sb):
                    for dd in range(NDD):
                        act_tile(nm, sb, dd)

                # ---- filler units (each holds one pA psum for its span) --
                def unit_qk(nm, sb, ot):
                    wsb, bias = (W8["q"], BQ) if nm == "q" else (W8["k"], BK)
                    ps = pA.tile([128, SB], F32, tag="A", name="psA")
                    for d in range(ND):
                        at = act_tile(nm, sb, d // 2)
                        nc.tensor.matmul(
                            ps[:], wsb[:, d, ot * 128:(ot + 1) * 128],
                            at[:, d % 2, :],
                            start=(d == 0), stop=(d == ND - 1))
                    if nm == "q":
                        nc.vector.tensor_scalar_add(
                            QT[ot][:, sb * SB:(sb + 1) * SB], ps[:],
                            bias[:, ot:ot + 1])
                    else:
                        # split into the two per-head padded K tiles
                        for hh in range(2):
                            h = 2 * ot + hh
                            r0 = (h % 2) * DK
                            nc.vector.tensor_scalar_add(
                                KP[h][r0:r0 + DK, sb * SB:(sb + 1) * SB],
                                ps[r0:r0 + DK, :],
                                bias[r0:r0 + DK, ot:ot + 1])

                def unit_v(sb, st):
                    ps = pA.tile([128, OL], F32, tag="A", name="psV")
                    for d in range(ND):
                        at = act_tile("v", sb, d // 2)
                        nc.tensor.matmul(
                            ps[:], at[:, d % 2, st * 128:(st + 1) * 128],
                            W8["v"][:, d, :],
                            start=(d == 0), stop=(d == ND - 1))
                    jb = sb * 4 + st
                    u, t = jb // 2, jb % 2
                    nc.vector.tensor_copy(
                        V8t[u][:, t, :, 0:DK],
                        ps[:].rearrange("p (h c) -> p h c", c=DK))
                    nc.vector.memset(V8t[u][:, t, :, DK:VS], 1.0)

                def unit_oproj(sb, jt2):
                    for jt in (2 * jt2, 2 * jt2 + 1):
                        ps = pA.tile([128, SB], F32, tag="A", name="psO")
                        for ot in range(2):
                            nc.tensor.matmul(
                                ps[:], WO[ot][:, jt * 128:(jt + 1) * 128],
                                XT[ot][:, sb * SB:(sb + 1) * SB],
                                start=(ot == 0), stop=(ot == 1))
                        ob = outp.tile([128, SB], BF16, tag="ob", name="ob")
                        nc.vector.tensor_copy(ob[:], ps[:])
                        nc.sync.dma_start(
                            out[jt * 128:(jt + 1) * 128,
                                sb * SB:(sb + 1) * SB], ob[:])

                def proj_units(sb):
                    return ([lambda ot=ot: unit_qk("q", sb, ot)
                             for ot in range(2)]
                            + [lambda ot=ot: unit_qk("k", sb, ot)
                               for ot in range(2)]
                            + [lambda st=st: unit_v(sb, st)
                               for st in range(4)])

                def oproj_units(sb):
                    return [lambda j=j: unit_oproj(sb, j) for j in range(4)]

                # ---- lead-in -------------------------------------------
                # critical-path DMAs first: the very first Q chain needs
                # only W8q[0:4] and the first q act tile
                nc.sync.dma_start(W8["q"][:, 0:4, :], wq8d[:, 0:4, :])
                act_tile("q", 0, 0)
                act_tile("q", 0, 1)
                nc.sync.dma_start(W8["q"][:, 4:8, :], wq8d[:, 4:8, :])
                act_tile("q", 0, 2)
                act_tile("q", 0, 3)
                nc.sync.dma_start(BQ[:], bqd[:, :])
                nc.sync.dma_start(W8["k"][:], wk8d[:, :, :])
                prefetch("k", 0)
                nc.sync.dma_start(BK[:], bkd[:, :])
                nc.sync.dma_start(W8["v"][:], wv8d[:, :, :])
                prefetch("v", 0)
                for i in range(nbias):
                    nc.sync.dma_start(TRI[i][:], trid[i])
                nc.sync.dma_start(ONE[:], oned[:, :])
                # PE clock warmup: ~4us of junk matmuls with NO deps at
                # all (QT is uninitialized here; results are discarded)
                for i in range(18):
                    wps = pA.tile([128, SB], F32, tag="A", name="warm")
                    nc.tensor.matmul(
                        wps[:], QT[0][:, 0:128], QT[1][:, 0:SB],
                        start=True, stop=True)
                for un in proj_units(0):
                    un()
                for t in range(2):
                    nc.sync.dma_start(WO[t][:], wod[t * 128:(t + 1) * 128, :])

                # filler plan: which units run inside each ib's supersteps.
                # Q/K of sb3 land in ib2 (scores of ib3 need them); V of
                # sb3 is deferred to ib3's first supersteps (only PV of
                # pairs 6-7 needs it) to keep the PE fed in the long
                # ACT-heavy ib3 window.
                pu3 = proj_units(3)
                fill_by_ib = {
                    0: proj_units(1),
                    1: proj_units(2),
                    2: pu3[:4] + oproj_units(0)[:2],
                    3: pu3[4:] + oproj_units(0)[2:] + oproj_units(1)
                       + oproj_units(2),
                }

                # ---- attention: per ib, two 2-head waves -----------------
                # PV work (and each head's normalization) drains as a
                # ROLLING queue ~2 supersteps behind the scores, crossing
                # wave and ib boundaries so the PE never waits on ACT at a
                # transition. pO bufs=2 holds exactly the two in-flight
                # waves' accumulators.
                pending = []  # (h, u, kms, Pt, first, last, Ops, norm_cb)

                def emit_pv(n):
                    for h, u, kms, Pt, first, last, Ops, norm in pending[:n]:
                        for z, (t, km) in enumerate(kms):
                            c0 = km * JB
                            nc.tensor.matmul(
                                Ops[:, c0:SB],
                                V8t[u][:, t, h, 0:VS],
                                Pt[:, t, c0:SB],
                                start=(first and z == 0),
                                stop=(last and z == len(kms) - 1))
                        if last and norm is not None:
                            norm()
                    del pending[:n]

                for ib in range(NSB):
                    jl = sched[ib]
                    pairs = [jl[i:i + 2] for i in range(0, len(jl), 2)]
                    npair = len(pairs)
                    fill = fill_by_ib[ib]
                    nss = 2 * npair  # supersteps this ib
                    fpos = 0
                    if ib < NSB - 1:
                        for key in [kk for kk in act_tiles
                                    if kk[1] < ib + 1]:
                            del act_tiles[key]
                        for nm in ("q", "k", "v"):
                            prefetch(nm, ib + 1)

                    for w in range(2):
                        hw = (2 * w, 2 * w + 1)
                        Ops = {h: pO.tile([VS, SB], F32, tag="O", name="Ops")
                               for h in hw}

                        def mk_norm(h, Opsh, ib=ib):
                            hp, bp = heads[h]

                            def norm():
                                ls = prr.tile([1, SB], F32, tag="tl",
                                              name="ls")
                                nc.vector.tensor_copy(ls[:], Opsh[DK:VS, :])
                                r0 = prr.tile([1, SB], F32, tag="r0",
                                              name="r0")
                                nc.vector.reciprocal_approx_fast(r0[:],
                                                                 ls[:])
                                rr = prr.tile([1, SB], F32R, tag="r",
                                              name="rr")
                                nc.vector.tensor_copy(rr[:], r0[:])
                                Rp = pA.tile([DK, SB], F32, tag="A",
                                             name="Rp")
                                nc.tensor.matmul(Rp[:], ONE[:], rr[:],
                                                 start=True, stop=True)
                                Rc = prc.tile([DK, SB], F32, tag="rc",
                                              name="Rc")
                                nc.vector.tensor_copy(Rc[:], Rp[:])
                                nc.vector.tensor_mul(
                                    XT[hp][bp:bp + DK,
                                           ib * SB:(ib + 1) * SB],
                                    Opsh[0:DK, :], Rc[:])
                            return norm

                        for pi, pair in enumerate(pairs):
                            assert all(jb // 2 == pair[0][0] // 2
                                       for (jb, _, _) in pair)
                            if pi == 0:
                                assert pair[0][1] == 0, \
                                    "first j-block must be visible from k=0"
                                # wave boundary: give the PE one filler unit
                                # BEFORE the scores so it rides over the
                                # S-slot turnaround without a clock drop
                                if fpos < len(fill):
                                    fill[fpos]()
                                    fpos += 1
                            for h in hw:
                                hp, bp = heads[h]
                                Sp = pS.tile([128, 2 * SB], F32, tag="S",
                                             name="Sp")
                                Sp3 = Sp[:].rearrange("p (z i) -> p z i",
                                                      z=2)
                                kmin = min(k0 for (_, k0, _) in pair)
                                for z, (jb, k0, sub_ops) in enumerate(pair):
                                    c0 = k0 * JB
                                    nc.tensor.matmul(
                                        Sp[:, z * SB + c0:(z + 1) * SB],
                                        KP[h][:, jb * JB:(jb + 1) * JB],
                                        QT[hp][:, ib * SB + c0:
                                               (ib + 1) * SB],
                                        start=True, stop=True)
                                Pt = pP.tile([128, 2, SB], BF16, tag="P",
                                             name="Pt")
                                nc.scalar.activation(
                                    Pt[:, :, kmin * JB:SB],
                                    Sp3[:, :, kmin * JB:SB], EXP,
                                    scale=0.125)
                                kms = []
                                for z, (jb, k0, sub_ops) in enumerate(pair):
                                    for k, (stt, bidx) in enumerate(sub_ops):
                                        if stt == "m" and k >= k0:
                                            nc.vector.memset(
                                                Pt[:, z,
                                                   k * JB:(k + 1) * JB],
                                                0.0)
                                        elif stt == "x":
                                            nc.gpsimd.tensor_mul(
                                                Pt[:, z,
                                                   k * JB:(k + 1) * JB],
                                                Pt[:, z,
                                                   k * JB:(k + 1) * JB],
                                                TRI[bidx][:])
                                    kms.append(
                                        (jb % 2,
                                         0 if pi == 0 and z == 0 else k0))
                                last = pi == npair - 1
                                pending.append(
                                    (h, pair[0][0] // 2, kms, Pt, pi == 0,
                                     last, Ops[h],
                                     mk_norm(h, Ops[h]) if last else None))
                            if len(pending) > 4:
                                emit_pv(2)
                            # filler: spread this ib's units over supersteps
                            # (front-loaded 3/4 in the last ib)
                            si = w * npair + pi
                            eff = (nss if ib < NSB - 1
                                   else max(1, (3 * nss) // 4))
                            emitted = False
                            while (fpos < len(fill)
                                   and fpos * eff <= (si + 1) * len(fill)):
                                fill[fpos]()
                                fpos += 1
                                emitted = True
                            if ib == NSB - 1 and fpos >= len(fill) \
                                    and not emitted and len(pending) <= 4:
                                # keep the PE clock hot through the
                                # ACT-bound stretch
                                wps = pA.tile([128, SB], F32, tag="A",
                                              name="warm2")
                                nc.tensor.matmul(
                                    wps[:], QT[0][:, 0:128], QT[1][:, 0:SB],
                                    start=True, stop=True)
                    while fpos < len(fill):
                        fill[fpos]()
                        fpos += 1
                # drain the remaining PV/normalization work
                while pending:
                    emit_pv(2)

                # ---- tail: out-projection for sb3 ------------------------
                for un in oproj_units(NSB - 1):
                    un()
    nc.finalize()
    return nc


def kernel(q, k, v, mask, wq, bq, wk, bk, wv, bv, wo, bo):
    global LAST_RUN
    q, k, v = (np.asarray(x, np.float32) for x in (q, k, v))
    wq, bq, wk, bk = (np.asarray(x, np.float32) for x in (wq, bq, wk, bk))
    wv, bv, wo, bo = (np.asarray(x, np.float32) for x in (wv, bv, wo, bo))
    mask2 = np.asarray(mask)[0, 0] != 0

    sched, bias_tiles = _classify_mask(mask2)
    nbias = len(bias_tiles)
    trib = np.stack(bias_tiles).astype(BF) if nbias else None

    bo_eff = (bo + wo @ bv).astype(np.float32)

    def act_layout(x):
        # [S, D] -> bf16 [NDD, 128, NSB, 2, SB] with paired d-block planes
        a8 = np.ascontiguousarray(x.T).astype(BF)          # [D, S]
        a8 = a8.reshape(NDD, 2, 128, NSB, SB)
        return np.ascontiguousarray(a8.transpose(0, 2, 3, 1, 4))

    q8s = [act_layout(q[b]) for b in range(B)]
    k8s = [act_layout(k[b]) for b in range(B)]
    v8s = [act_layout(v[b]) for b in range(B)]

    def w_layout(w):
        # rows [OL, D] slice -> bf16 [128, ND, OL]
        w8 = np.ascontiguousarray(w.T).astype(BF)          # [D, OL]
        return np.ascontiguousarray(
            w8.reshape(ND, 128, OL).transpose(1, 0, 2))

    wq8s, wk8s, wv8s, wos, bqs, bks = [], [], [], [], [], []
    for g in range(GROUPS):
        rows = slice(g * OL, (g + 1) * OL)
        wq8s.append(w_layout(wq[rows]))
        wk8s.append(w_layout(wk[rows]))
        wv8s.append(w_layout(wv[rows]))
        wos.append(_round_f32r(wo[:, rows].T))
        bqs.append(np.ascontiguousarray(
            bq[rows].reshape(2, 128).T.astype(np.float32)))
        bks.append(np.ascontiguousarray(
            bk[rows].reshape(2, 128).T.astype(np.float32)))

    in_maps = []
    for c in range(NCORES):
        b, g = c // GROUPS, c % GROUPS
        m = {
            "q8": q8s[b], "k8": k8s[b], "v8": v8s[b],
            "wq8": wq8s[g], "wk8": wk8s[g], "wv8": wv8s[g],
            "woT": wos[g], "bq": bqs[g], "bk": bks[g],
            "ones1": np.ones((1, DK), np.float32),
        }
        if nbias:
            m["tri"] = trib
        in_maps.append(m)

    nc = _build(sched, nbias)
    res = run_bass_kernel_spmd(nc, in_maps, core_ids=list(range(NCORES)))
    LAST_RUN = res
    if res.exec_time_ns is not None:
        print(f"HW exec time: {res.exec_time_ns} ns")

    outp = np.zeros((B, S, D), np.float32)
    for c in range(NCORES):
        b = c // GROUPS
        outp[b] += np.asarray(res.results[c]["out"]).astype(np.float32).T
    outp += bo_eff
    return outp
